# revision 1
# baseline (speedup 1.0000x reference)
"""Bass/Trainium2 kernel for nn_CasualSelfAttention (B=4, T=2048, D=1024, H=16, dk=64).

Sharding: batch (4) x head-group (2) = 8 cores. Each core computes 8 heads of one
batch element end-to-end (QKV projections, attention, WO partial product); the
host sums the two head-group partials per batch and folds the free-dim biases.

All big matmuls run in float32r (full PE rate, ~tf32 precision). Softmax sums are
obtained by augmenting V with a ones column (M=65 PV matmuls); normalization uses
a DVE reciprocal plus a K=1 PE broadcast matmul.
"""
import sys
import os

sys.path.insert(0, '/opt/trn_rl_repo')

import numpy as np
import ml_dtypes
import orjson

import concourse.bass as bass
import concourse.tile as tile
import concourse.mybir as mybir
from concourse.bass_utils import run_bass_kernel_spmd

# ---------------------------------------------------------------- waitsplit
# The walrus build in this container accepts at most ONE semaphore wait per
# engine instruction.  Tile emits multi-wait sync_info; split the extras into
# single-wait NoOps on the same engine stream (in-order => semantically equal).
_ws_counter = [0]


_SELF_WAIT_ENGINES = ("Activation", "DVE")


def _split_instruction_waits(inst, out_list):
    si = inst.get("sync_info")
    if not si or not si.get("on_wait"):
        out_list.append(inst)
        return
    waits = si["on_wait"]
    # ACT/DVE execute strictly in order, so a compute instruction's wait on
    # its OWN engine's semaphore (slot-reuse WAW vs an older instruction on
    # the same engine) is always already satisfied — drop it instead of
    # spending a NoOp dispatch on the bottleneck ACT stream.
    eng = inst.get("engine")
    if (eng in _SELF_WAIT_ENGINES
            and inst.get("opcode") not in ("Drain", "EventSemaphore", "NoOp")):
        kept = [w for w in waits
                if w.get("ant_name", "").rsplit("_", 1)[0] != eng]
        if kept != waits:
            si = dict(si)
            si["on_wait"] = kept
            inst = dict(inst)
            inst["sync_info"] = si
            waits = kept
    if len(waits) <= 1:
        out_list.append(inst)
        return
    for w in waits[:-1]:
        _ws_counter[0] += 1
        out_list.append({
            "debug": inst.get("debug", 0),
            "engine": inst.get("engine"),
            "ins": [],
            "name": f"I-wsplit-{_ws_counter[0]}",
            "opcode": "NoOp",
            "outs": [],
            "sync_info": {"on_update": [], "on_wait": [w]},
        })
    si = dict(si)
    si["on_wait"] = [waits[-1]]
    inst = dict(inst)
    inst["sync_info"] = si
    out_list.append(inst)


def fix_multiwait_json(bir_bytes):
    d = orjson.loads(bir_bytes)
    for fn in d["functions"]:
        for bb in fn["blocks"]:
            new = []
            for inst in bb["instructions"]:
                _split_instruction_waits(inst, new)
            bb["instructions"] = new
    return orjson.dumps(d)


class WaitSplitBass(bass.Bass):
    def to_json_bytes(self):
        return fix_multiwait_json(super().to_json_bytes())


# ---------------------------------------------------------------- kernel build
P = 128
B, T, D = 4, 2048, 1024
NH_LOC = 8            # heads per core
NP = NH_LOC // 2      # head pairs per core
DK = 64
DC = D // P           # 8 d_model chunks
SC = T // P           # 16 s-chunks
NTB = T // 512        # 4 t-blocks
f32 = mybir.dt.float32
f32r = mybir.dt.float32r
bf16 = mybir.dt.bfloat16
AF = mybir.ActivationFunctionType
MULT = mybir.AluOpType.mult

_nc_cache = [None]


def build_nc():
    if _nc_cache[0] is not None:
        return _nc_cache[0]
    nc = WaitSplitBass()
    xq_t = nc.dram_tensor("xq_t", [D, T], f32r, kind="ExternalInput")
    xk_t = nc.dram_tensor("xk_t", [D, T], f32r, kind="ExternalInput")
    xv_t = nc.dram_tensor("xv_t", [D, T], f32r, kind="ExternalInput")
    wq_t = nc.dram_tensor("wq_t", [D, 512], f32r, kind="ExternalInput")
    wk_t = nc.dram_tensor("wk_t", [D, 512], f32r, kind="ExternalInput")
    wv_t = nc.dram_tensor("wv_t", [D, 512], f32r, kind="ExternalInput")
    wo_t = nc.dram_tensor("wo_t", [512, D], f32r, kind="ExternalInput")
    bq_d = nc.dram_tensor("bq_d", [P, NP], f32, kind="ExternalInput")
    bk_d = nc.dram_tensor("bk_d", [P, NP], f32, kind="ExternalInput")
    ones_d = nc.dram_tensor("ones_d", [1, 64], f32r, kind="ExternalInput")
    out = nc.dram_tensor("out", [T, D], f32, kind="ExternalOutput")

    with tile.TileContext(nc) as tc:
        with tc.tile_pool(name="persist", bufs=1) as persist, \
             tc.tile_pool(name="psProj", bufs=2, space="PSUM") as psProj, \
             tc.tile_pool(name="psS", bufs=2, space="PSUM") as psS, \
             tc.tile_pool(name="psPV", bufs=1, space="PSUM") as psPV:

            # ---- persistent tiles ----
            qT2 = [persist.tile([P, T], f32r, tag=f"qT2_{p}", name=f"qT2_{p}")
                   for p in range(NP)]
            kT2 = [persist.tile([P, T], f32r, tag=f"kT2_{p}", name=f"kT2_{p}")
                   for p in range(NP)]
            V_aug = persist.tile([P, SC, NH_LOC, 65], f32r, name="V_aug")
            nc.vector.memset(V_aug[:, :, :, 64].bitcast(f32), 1.0)
            bq_s = persist.tile([P, NP], f32, name="bq_s")
            nc.sync.dma_start(bq_s[:], bq_d[:])
            bk_s = persist.tile([P, NP], f32, name="bk_s")
            nc.sync.dma_start(bk_s[:], bk_d[:])
            ones64 = persist.tile([1, 64], f32r, name="ones64")
            nc.sync.dma_start(ones64[:], ones_d[:])

            # ---- phase A: projections (weights + X^T streamed per chunk) ----
            # 4 concurrent psum groups: 2 slots borrowed from the (idle)
            # scores pool + 2 from psProj.
            def alloc4(stem):
                ps = [psS.tile([P, 1024], f32, tag="scores",
                               name=f"{stem}_s{j}")[:, 0:512] for j in range(2)]
                ps += [psProj.tile([P, 512], f32, tag="proj",
                                   name=f"{stem}_p{j}") for j in range(2)]
                return ps

            ctx_00 = persist.tile([P, 512], f32r, name="ctx_00")
            with tc.tile_pool(name="pearly", bufs=2) as pearly, \
                 tc.tile_pool(name="wpool", bufs=1) as wpool, \
                 tc.tile_pool(name="xpool", bufs=8) as xpool:
                wq = wpool.tile([P, DC, 512], f32r, tag="wq", name="wq")
                wk = wpool.tile([P, DC, 512], f32r, tag="wk", name="wk")
                wv = wpool.tile([P, DC, 512], f32r, tag="wv", name="wv")
                for c in range(DC):
                    nc.sync.dma_start(wq[:, c], wq_t[c * P:(c + 1) * P, :])
                    nc.sync.dma_start(wk[:, c], wk_t[c * P:(c + 1) * P, :])
                    nc.sync.dma_start(wv[:, c], wv_t[c * P:(c + 1) * P, :])

                for tb in range(NTB):
                    ts_ = slice(tb * 512, (tb + 1) * 512)
                    # q then k: 4 pair-groups, chunk-outer accumulation
                    for qk, (w_t, xdram, b_s, dst) in enumerate(
                            ((wq, xq_t, bq_s, qT2), (wk, xk_t, bk_s, kT2))):
                        ps4 = alloc4(f"psqk{tb}_{qk}")
                        for c in range(DC):
                            x_c = xpool.tile([P, 512], f32r, tag=f"x{qk}",
                                             name=f"x{qk}_{tb}_{c}")
                            nc.sync.dma_start(x_c[:], xdram[c * P:(c + 1) * P, ts_])
                            for p in range(NP):
                                nc.tensor.matmul(
                                    ps4[p][:], w_t[:, c, p * P:(p + 1) * P],
                                    x_c[:], start=(c == 0), stop=(c == DC - 1))
                        for p in range(NP):
                            nc.vector.tensor_scalar_add(
                                dst[p][:, ts_], ps4[p][:], b_s[:, p:p + 1])
                    # v: 4 t-tile groups, chunk-outer
                    ps4 = alloc4(f"psv{tb}")
                    for c in range(DC):
                        xv_c = xpool.tile([P, 512], f32r, tag="xv",
                                          name=f"xv_{tb}_{c}")
                        nc.sync.dma_start(xv_c[:], xv_t[c * P:(c + 1) * P, ts_])
                        for ti in range(4):
                            nc.tensor.matmul(
                                ps4[ti][:], xv_c[:, ti * P:(ti + 1) * P],
                                wv[:, c], start=(c == 0), stop=(c == DC - 1))
                    for ti in range(4):
                        tt = tb * 4 + ti
                        nc.vector.tensor_copy(
                            V_aug[:, tt, :, 0:64],
                            ps4[ti][:].rearrange("p (h d) -> p h d", d=64))

                    # early attention for (t-block 0, pair 0): its PV psum is
                    # idle during phase A and every dep of s-chunk quarter tb
                    # is produced by A(tb) — run it here so ACT starts ~100us
                    # earlier instead of idling through all projections.
                    if tb == 0:
                        pv_e0 = psPV.tile([65, 512], f32, tag="pv0",
                                          name="pv0_0_0")
                        pv_e1 = psPV.tile([65, 512], f32, tag="pv1",
                                          name="pv1_0_0")
                    for sc in range(4 * tb, 4 * tb + 4):
                        ss = slice(sc * P, (sc + 1) * P)
                        ps_s = psS.tile([P, 1024], f32, tag="scores",
                                        name=f"ps_s_0_0_{sc}")
                        nc.tensor.matmul(
                            ps_s[:, 0:512], qT2[0][0:64, ss],
                            kT2[0][0:64, 0:512], start=True, stop=True,
                            tile_position=(0, 0))
                        nc.tensor.matmul(
                            ps_s[:, 512:1024], qT2[0][64:128, ss],
                            kT2[0][64:128, 0:512], start=True, stop=True,
                            tile_position=(64, 0))
                        p_t = pearly.tile([P, 1024], f32r, tag="pe",
                                          name=f"pe_{sc}")
                        nc.scalar.activation(p_t[:], ps_s[:], AF.Exp,
                                             scale=0.125)
                        nc.tensor.matmul(
                            pv_e0[:], V_aug[:, sc, 0, :], p_t[:, 0:512],
                            start=(sc == 0), stop=(sc == SC - 1))
                        nc.tensor.matmul(
                            pv_e1[:], V_aug[:, sc, 1, :], p_t[:, 512:1024],
                            start=(sc == 0), stop=(sc == SC - 1))

            # ---- phases B+C per t-block ----
            with tc.tile_pool(name="ppool", bufs=5) as ppool, \
                 tc.tile_pool(name="rbpool", bufs=2) as rbpool, \
                 tc.tile_pool(name="ctxpool", bufs=2) as ctxpool, \
                 tc.tile_pool(name="wopool", bufs=1) as wopool, \
                 tc.tile_pool(name="opool", bufs=3) as opool:
                    wo = wopool.tile([P, NP, D], f32r, name="wo")
                    for p in range(NP):
                        nc.sync.dma_start(wo[:, p], wo_t[p * P:(p + 1) * P, :])

                    def flush_evac(pend):
                        # normalize pair into its ctx tile:
                        # ctx[h] = pv[h][0:64] * bcast(1 / pv[h][64])
                        tb, p, pv0, pv1, ctx_p = pend
                        for h, pv in ((0, pv0), (1, pv1)):
                            r_t = rbpool.tile([1, 512], f32r, tag="r",
                                              name=f"r_{tb}_{p}_{h}")
                            with nc.allow_low_precision(reason="softmax recip"):
                                nc.vector.reciprocal(r_t[:], pv[64:65, :])
                            ps_rb = psProj.tile([64, 512], f32, tag="proj",
                                                name=f"ps_rb_{tb}_{p}_{h}")
                            nc.tensor.matmul(ps_rb[:], ones64[:], r_t[:],
                                             start=True, stop=True)
                            rb_s = rbpool.tile([64, 512], f32, tag="rb",
                                               name=f"rb_{tb}_{p}_{h}")
                            nc.vector.tensor_copy(rb_s[:], ps_rb[:])
                            nc.vector.tensor_tensor(
                                ctx_p[h * 64:(h + 1) * 64, :],
                                pv[0:64, :], rb_s[:], MULT)

                    def emit_wo_chunk(wtb, wctx, ti, ob):
                        # one [128t, 512o] WO output tile of t-block wtb
                        ps_o = psProj.tile([P, 512], f32, tag="proj",
                                           name=f"ps_o_{wtb}_{ti}_{ob}")
                        for p in range(NP):
                            nc.tensor.matmul(
                                ps_o[:], wctx[p][:, ti * P:(ti + 1) * P],
                                wo[:, p, ob * 512:(ob + 1) * 512],
                                start=(p == 0), stop=(p == NP - 1))
                        o_t = opool.tile([P, 512], f32, tag="o",
                                         name=f"o_{wtb}_{ti}_{ob}")
                        nc.vector.tensor_copy(o_t[:], ps_o[:])
                        nc.sync.dma_start(
                            out[wtb * 512 + ti * P: wtb * 512 + (ti + 1) * P,
                                ob * 512:(ob + 1) * 512], o_t[:])

                    # (t-block 0, pair 0) already ran during phase A; seed its
                    # deferred evacuation so pair 1's sc==2 flush handles it.
                    pending = (0, 0, pv_e0, pv_e1, ctx_00)
                    pending_wo = None     # (tb, ctx_tb) whose WO is deferred
                    for tb in range(NTB):
                        ts_ = slice(tb * 512, (tb + 1) * 512)
                        ctx_tb = [ctx_00] if tb == 0 else []
                        for p in range(NP):
                            if tb == 0 and p == 0:
                                continue
                            # -- B: attention for (pair p, t-block tb) --
                            pv0 = psPV.tile([65, 512], f32, tag="pv0",
                                            name=f"pv0_{tb}_{p}")
                            pv1 = psPV.tile([65, 512], f32, tag="pv1",
                                            name=f"pv1_{tb}_{p}")
                            # Defer the previous pair's PV-psum evacuation (and
                            # the previous t-block's WO chunks) past this
                            # pair's first score/exp groups so ACT stays fed
                            # while PE runs the evac/WO work in its slack.
                            stash = []
                            for sc in range(SC):
                                ss = slice(sc * P, (sc + 1) * P)
                                ps_s = psS.tile([P, 1024], f32, tag="scores",
                                                name=f"ps_s_{tb}_{p}_{sc}")
                                nc.tensor.matmul(
                                    ps_s[:, 0:512], qT2[p][0:64, ss],
                                    kT2[p][0:64, ts_], start=True, stop=True,
                                    tile_position=(0, 0))
                                nc.tensor.matmul(
                                    ps_s[:, 512:1024], qT2[p][64:128, ss],
                                    kT2[p][64:128, ts_], start=True, stop=True,
                                    tile_position=(64, 0))
                                p_t = ppool.tile([P, 1024], f32r, tag="p",
                                                 name=f"p_{tb}_{p}_{sc}")
                                nc.scalar.activation(p_t[:], ps_s[:], AF.Exp,
                                                     scale=0.125)
                                if pending is not None and sc < 2:
                                    stash.append((sc, p_t))
                                    continue
                                if pending is not None and sc == 2:
                                    flush_evac(pending)
                                    pending = None
                                for s0, pt0 in stash:
                                    nc.tensor.matmul(
                                        pv0[:], V_aug[:, s0, 2 * p, :],
                                        pt0[:, 0:512],
                                        start=(s0 == 0), stop=False)
                                    nc.tensor.matmul(
                                        pv1[:], V_aug[:, s0, 2 * p + 1, :],
                                        pt0[:, 512:1024],
                                        start=(s0 == 0), stop=False)
                                stash = []
                                nc.tensor.matmul(
                                    pv0[:], V_aug[:, sc, 2 * p, :],
                                    p_t[:, 0:512],
                                    start=(sc == 0), stop=(sc == SC - 1))
                                nc.tensor.matmul(
                                    pv1[:], V_aug[:, sc, 2 * p + 1, :],
                                    p_t[:, 512:1024],
                                    start=(sc == 0), stop=(sc == SC - 1))
                                # sprinkle the previous t-block's 8 WO
                                # chunks across pairs 0-1, every other sc,
                                # to stay under the ACT rate per slot
                                if (pending_wo is not None and p <= 1
                                        and 2 <= sc <= 9 and (sc % 2) == 0):
                                    widx = p * 4 + (sc - 2) // 2
                                    emit_wo_chunk(pending_wo[0], pending_wo[1],
                                                  widx // 2, widx % 2)
                                    if widx == 7:
                                        pending_wo = None
                            ctx_p = ctxpool.tile([P, 512], f32r, tag=f"ctx{p}",
                                                 name=f"ctx_{tb}_{p}")
                            pending = (tb, p, pv0, pv1, ctx_p)
                            ctx_tb.append(ctx_p)
                        pending_wo = (tb, ctx_tb)

                    # tail: last pair's evac + last t-block's WO
                    if pending is not None:
                        flush_evac(pending)
                        pending = None
                    if pending_wo is not None:
                        for ti in range(4):
                            for ob in range(2):
                                emit_wo_chunk(pending_wo[0], pending_wo[1],
                                              ti, ob)
                        pending_wo = None
    _nc_cache[0] = nc
    return nc


# ---------------------------------------------------------------- host side
def make_in_maps(keys, queries, values, WK_w, WK_b, WQ_w, WQ_b, WV_w, WV_b, WO_w):
    keys = np.asarray(keys, dtype=np.float32)
    queries = np.asarray(queries, dtype=np.float32)
    values = np.asarray(values, dtype=np.float32)
    xq_b = [np.ascontiguousarray(queries[b].T) for b in range(B)]
    xk_b = [np.ascontiguousarray(keys[b].T) for b in range(B)]
    xv_b = [np.ascontiguousarray(values[b].T) for b in range(B)]
    ones = np.ones((1, 64), np.float32)
    in_maps = []
    for c in range(8):
        b, g = c // 2, c % 2
        sl = slice(512 * g, 512 * (g + 1))
        in_maps.append({
            "xq_t": xq_b[b], "xk_t": xk_b[b], "xv_t": xv_b[b],
            "wq_t": np.ascontiguousarray(np.asarray(WQ_w, np.float32)[sl, :].T),
            "wk_t": np.ascontiguousarray(np.asarray(WK_w, np.float32)[sl, :].T),
            "wv_t": np.ascontiguousarray(np.asarray(WV_w, np.float32)[sl, :].T),
            "wo_t": np.ascontiguousarray(np.asarray(WO_w, np.float32)[:, sl].T),
            "bq_d": np.ascontiguousarray(
                np.asarray(WQ_b, np.float32)[sl].reshape(NP, P).T),
            "bk_d": np.ascontiguousarray(
                np.asarray(WK_b, np.float32)[sl].reshape(NP, P).T),
            "ones_d": ones,
        })
    return in_maps


def kernel(keys, queries, values, pad_mask, WK_w, WK_b, WQ_w, WQ_b, WV_w, WV_b,
           WO_w, WO_b):
    nc = build_nc()
    in_maps = make_in_maps(keys, queries, values, WK_w, WK_b, WQ_w, WQ_b,
                           WV_w, WV_b, WO_w)
    res = run_bass_kernel_spmd(nc, in_maps, list(range(8)))
    # free-dim biases folded on host: WO_b directly; WV_b exactly via
    # sum_g (WV_b_g @ WO_g^T) = WV_b @ WO_w^T  (attention rows sum to 1).
    bias = (np.asarray(WO_b, np.float64)
            + np.asarray(WV_b, np.float64) @ np.asarray(WO_w, np.float64).T)
    out = np.empty((B, T, D), np.float32)
    for b in range(B):
        out[b] = (res.results[2 * b]["out"].astype(np.float64)
                  + res.results[2 * b + 1]["out"].astype(np.float64)
                  + bias).astype(np.float32)
    return out

